# revision 11
# baseline (speedup 1.0000x reference)
"""DenseGIN (3-layer, dense adjacency) Trainium2 Bass kernel, 8-core SPMD. v4.

Problem: x:(4,4096,2,32) f32, adj:(4,4096,4096) f32 binary, mask:(4,4096) bool.
Per layer l: agg = (adj+I) @ xf ; h = relu(agg@Wa+ba); h = BN(h); h = h@Wb+bb;
x = mask*h ; between layers an outer BN is applied at masked nodes.

Sharding: 8 cores = (batch b, node-half). Core (2b+h) owns output nodes
[h*2048,(h+1)*2048) of batch b.

v4 design (on top of v3's fp8 adjacency residency + chunk pipelining):
- The aggregation is linear, so per core
      agg = B_own @ y_own + B_peer @ y_peer
          = (B_own - B_peer) @ y_own + B_peer @ (y_own + y_peer)
  with B = (adj+I).T column slice.  The host bakes D = B_own - B_peer
  (entries {-1,0,1}, exact in fp8e4) and P = B_peer per core, which removes
  the even/odd parity asymmetry from the program: the inter-core exchange
  becomes an AllReduce(add) of y (s = y_own + y_peer), and 16 of the 32
  aggregation matmuls per chunk ("D-group") depend only on locally-computed
  activations.  All four D-groups (64 MMs, ~14us) run at each layer boundary
  while the AllReduces complete -> no PE stall, no HAM p-state drop.
- Mask/bias folding: xst carries y = m (.) z only (single scalar op from
  PSUM).  The dropped +D bias enters the next layer's aggregation as a
  rank-1 update deg (x) D (one K=1 matmul per chunk), with
  deg_j = sum_s (adj+I)[j,s] m_s precomputed on host (integer, bf16-exact).
- Layer 2's MLP2 is flipped (wb stationary, h1 moving): one [64,512] matmul
  per chunk instead of 4 small ones; final mask + bias applied on host.
- All MLP weights/activations in bf16 (fp32 matmuls run at 1/4 rate).
- Layer 0 packs x as [xh|xl] bf16 hi/lo in 128 stationary columns with wa0
  stacked [Wa;Wa] so the hi+lo reduction happens inside the MLP1 matmul;
  the host also prepares s0 = x_own + x_peer hi/lo so layer 0 uses the same
  D/P slab structure.
- Engine queues: sync = bulk input DMA + ar_in/output stores; gpsimd =
  consts + AllReduce triggers + s_sb loads (each load only waits on a CC
  already ahead of the next trigger in the serial CC stream); scalar/vector
  split the epilogue ops.
"""

import sys

if "/opt/trn_rl_repo" not in sys.path:  # PYTHONPATH normally provides it
    sys.path.insert(0, "/opt/trn_rl_repo")

import contextlib
import ctypes
import types

import numpy as np
import ml_dtypes

import concourse.bass as bass
import concourse.tile as tile
from concourse import mybir
from concourse.vector_clock import ScopedClock
import concourse.bass_utils as bass_utils
from concourse.bass_utils import run_bass_kernel_spmd

# ---------------------------------------------------------------------------
# Workaround: the walrus build in this container rejects instructions with
# more than one sem wait ("Too many sync wait commands").  Tile's final drain
# attaches one wait per live semaphore; split them across chained SP drains.
_MAX_WAITS_PER_INST = 1


def _patched_drain_and_barrier(self, tick_clock, wait_clock):
    nc = self.nc
    drain_inst = nc.sync.drain()
    wait_clock.add_sem_waits(drain_inst.ins, ScopedClock({None: tick_clock.global_clock}))
    si = drain_inst.ins.sync_info
    waits = list(si.on_wait or [])
    if len(waits) > _MAX_WAITS_PER_INST:
        si.on_wait = waits[:_MAX_WAITS_PER_INST]
        rest = waits[_MAX_WAITS_PER_INST:]
        for i in range(0, len(rest), _MAX_WAITS_PER_INST):
            extra = nc.sync.drain()
            extra.ins.sync_info = mybir.SyncInfo(
                on_wait=rest[i : i + _MAX_WAITS_PER_INST], on_update=[]
            )
    nc.all_engine_barrier()
    assert self.sems is not None
    popped = nc._tile_sem_poison_stack.pop()
    assert popped is self._sem_poison
    nc.clear_and_free_semaphores(list(self.sems.allocated().values()))
    nc.all_engine_barrier()


tile.TileContext._drain_and_barrier = _patched_drain_and_barrier


def _legalize_sync_waits(nc, max_waits=_MAX_WAITS_PER_INST):
    """Split instructions carrying more than ``max_waits`` sem waits.

    Engine sequencers process their instruction stream in order and execute
    sem waits before dispatch, so hoisting excess waits onto NoOps placed
    just before the instruction (same engine) is semantics-preserving.
    """
    n_split = 0
    for fn in nc.m.functions:
        for blk in fn.blocks:
            insts = blk.instructions
            i = 0
            while i < len(insts):
                inst = insts[i]
                si = inst.sync_info
                waits = list(si.on_wait) if si and si.on_wait else []
                if len(waits) > max_waits:
                    extra, keep = waits[:-max_waits], waits[-max_waits:]
                    si.on_wait = keep
                    pos = i
                    for j in range(0, len(extra), max_waits):
                        nop = mybir.InstNoOp(name=f"I-lsw{n_split}-{j}", ins=[], outs=[])
                        nop.engine = inst.engine
                        nop.sync_info = mybir.SyncInfo(
                            on_wait=extra[j : j + max_waits], on_update=[]
                        )
                        insts.insert(pos, nop)
                        pos += 1
                        i += 1
                    n_split += 1
                i += 1
    return n_split


# ---------------------------------------------------------------------------
# NTFF profiling hook (antenv.axon_hooks is absent in this image).  Only used
# when run() is called with trace=True; registering it is harmless otherwise.
def _ntff_profile_via_ctypes(so_path):
    try:
        lib = ctypes.CDLL(so_path)
    except OSError:
        return None
    if not hasattr(lib, "axon_start_nrt_profile"):
        return None
    lib.axon_start_nrt_profile.argtypes = [ctypes.POINTER(ctypes.c_int64), ctypes.c_size_t]
    lib.axon_start_nrt_profile.restype = ctypes.c_int64
    lib.axon_stop_nrt_profile.argtypes = [ctypes.c_char_p]
    lib.axon_stop_nrt_profile.restype = ctypes.c_int64

    @contextlib.contextmanager
    def _hook(output_dir, device_ids):
        import jax

        jax.devices()
        if device_ids:
            ids = (ctypes.c_int64 * len(device_ids))(*device_ids)
            rc = lib.axon_start_nrt_profile(ids, len(device_ids))
        else:
            rc = lib.axon_start_nrt_profile(None, 0)
        if rc != 0:
            raise RuntimeError(f"axon_start_nrt_profile rc={rc}")
        try:
            yield
        finally:
            n = lib.axon_stop_nrt_profile(str(output_dir).encode())
            print(f"ntff profile: {n} file(s) written to {output_dir}", file=sys.stderr)

    return _hook


if "antenv.axon_hooks" not in sys.modules:
    _hooks_mod = types.ModuleType("antenv.axon_hooks")
    _hook_inst = _ntff_profile_via_ctypes("/opt/axon/libaxon_pjrt.so")
    _hooks_mod.get_axon_ntff_profile_hook = lambda: _hook_inst
    sys.modules["antenv.axon_hooks"] = _hooks_mod
bass_utils.upload_artifacts = lambda tmpdir: f"local:{tmpdir}"

# ---------------------------------------------------------------------------
B, N, K, C_IN, H, C_OUT = 4, 4096, 2, 32, 64, 32
BN_EPS = 1e-5
N_CORES = 8
HALF = N // 2          # 2048 output nodes per core
NT = 32                # 32 contraction slot tiles: 16 D (own) + 16 P (peer/sum)
KC_OUT = [K * H, K * H, K * C_OUT]   # flat output channels per layer: 128,128,64

BF16 = ml_dtypes.bfloat16
FP8 = ml_dtypes.float8_e4m3  # == mybir float8e4 (TRN FP8_EXP4); {-1,0,1} exact

PAIRS = [[0, 1], [2, 3], [4, 5], [6, 7]]

_PROGRAM_CACHE = {}


def _build_program(n_layers=3, use_cc=True):
    """Build the SPMD Bass/Tile program (identical on all 8 cores)."""
    nc = bass.Bass("TRN2", target_bir_lowering=False, debug=False, num_devices=N_CORES)
    dt = mybir.dt

    # chunk-major adjacency slabs: row kc*128+p, col i*512+j =
    #   D[i*128+p, kc*512+j]          for i < 16   (D = B_own - B_peer)
    #   P[(i-16)*128+p, kc*512+j]     for i >= 16  (P = B_peer)
    # Chunk kc's slab is one contiguous [128, 16384] block, D half first.
    adjc_d = nc.dram_tensor("adjc", [4 * 128, NT * 512], dt.float8e4, kind="ExternalInput").ap()
    # layer-0 x, packed [xh | xl] bf16, block-permuted: col i*128+c = slot-tile
    # i's packed channel c.  Slot tiles 0-15 = own x, 16-31 = s0 = x_own+x_peer.
    x0_d = nc.dram_tensor("x0p", [128, NT * 128], dt.bfloat16, kind="ExternalInput").ap()
    mask_d = nc.dram_tensor("mask_cols", [128, 16], dt.float32, kind="ExternalInput").ap()
    # deg/drow are logically 1-row; zero-padded to K=32 partitions so the
    # rank-1 matmul runs as a standard 32-row tile (no reliance on HW
    # zero-padding of sub-tile moving operands).
    deg_d = nc.dram_tensor("deg_row", [32, HALF], dt.bfloat16, kind="ExternalInput").ap()
    drow_d = nc.dram_tensor("drow", [32, 256], dt.bfloat16, kind="ExternalInput").ap()
    dummy_d = nc.dram_tensor("cc_dummy", [2, 16], dt.float32, kind="ExternalInput").ap()
    # Wa/Wb are stored block-diagonally over the K=2 slice structure so each
    # MLP stage is a single full-partition matmul with base_partition 0
    # (partition-offset matmul operands crash at runtime on this stack).
    # wa0 additionally stacks the block twice (hi/lo reduction in the MM).
    wa_d = [
        nc.dram_tensor(f"wa{l}", [128, 2 * H], dt.bfloat16, kind="ExternalInput").ap()
        for l in range(3)
    ]
    wb_d = [
        nc.dram_tensor(f"wb{l}", [2 * H, KC_OUT[l]], dt.bfloat16, kind="ExternalInput").ap()
        for l in range(3)
    ]
    ba_d = [
        nc.dram_tensor(f"ba{l}", [128, 1], dt.float32, kind="ExternalInput").ap()
        for l in range(3)
    ]
    # final output: flipped layout [flat channel, node] (host transposes)
    out_d = nc.dram_tensor("out", [KC_OUT[2], HALF], dt.float32, kind="ExternalOutput").ap()

    with tile.TileContext(nc) as tc:
        with (
            tc.tile_pool(name="const", bufs=1) as cpool,
            tc.tile_pool(name="xst", bufs=1) as xstpool,
            tc.tile_pool(name="work", bufs=3) as wpool,
            tc.tile_pool(name="ps_agg", bufs=1, space="PSUM") as ps_agg,
            tc.tile_pool(name="ps_mlp", bufs=2, space="PSUM") as ps_mlp,
            tc.tile_pool(name="dram", bufs=2, space="DRAM") as dpool,
        ):
            # --- HAM warmup: dummy matmuls prime the PE clock toward 8/8
            # while the head DMAs stream ---
            wu_lhs = cpool.tile([128, 128], dt.bfloat16, tag="wu_lhs")
            wu_rhs = cpool.tile([128, 512], dt.bfloat16, tag="wu_rhs")
            nc.gpsimd.memset(wu_lhs[:], 0.0)
            nc.gpsimd.memset(wu_rhs[:], 0.0)
            wu_ps = ps_mlp.tile([128, 512], dt.float32, tag="h1")
            for _ in range(8):
                nc.tensor.matmul(wu_ps[:], wu_lhs[:], wu_rhs[:], start=True, stop=True)

            # --- dummy AllReduce first: every core reaches this trigger
            # within ~1us of launch (no data deps), so the one-time CC
            # bootstrap/rendezvous runs against launch skew instead of
            # layer-0 compute progress ---
            if use_cc:
                dummy_in = dpool.tile([2, 16], dt.float32, tag="cc_warm_in", bufs=1)
                dummy_out = dpool.tile([2, 16], dt.float32, tag="cc_warm_out", bufs=1)
                nc.gpsimd.dma_start(dummy_in[:], dummy_d[:])
                nc.gpsimd.collective_compute(
                    "AllReduce",
                    mybir.AluOpType.add,
                    replica_groups=PAIRS,
                    ins=[dummy_in.opt()],
                    outs=[dummy_out.opt()],
                )

            # --- bulk inputs on the sync queue in consumption order:
            # x0-own, chunk-0 D half (quarters), x0-s, chunk-0 P half
            # (quarters), then chunks 1-3 as D/P halves ---
            x0_sb = [
                cpool.tile([128, 16 * 128], dt.bfloat16, tag=f"x0h{h}", name=f"x0_{h}")
                for h in range(2)
            ]
            adjc0_sb = [
                cpool.tile([128, 8 * 512], dt.float8e4, tag=f"adjc0q{q}", name=f"adjc0_{q}")
                for q in range(4)
            ]
            adjc_sb = {
                kc: [
                    cpool.tile([128, 16 * 512], dt.float8e4, tag=f"adjc{kc}h{h}", name=f"adjc_{kc}_{h}")
                    for h in range(2)
                ]
                for kc in range(1, 4)
            }
            nc.sync.dma_start(x0_sb[0][:], x0_d[:, 0:2048])
            for q in range(2):
                nc.sync.dma_start(adjc0_sb[q][:], adjc_d[0:128, q * 4096 : (q + 1) * 4096])
            nc.sync.dma_start(x0_sb[1][:], x0_d[:, 2048:4096])
            for q in range(2, 4):
                nc.sync.dma_start(adjc0_sb[q][:], adjc_d[0:128, q * 4096 : (q + 1) * 4096])
            for kc in range(1, 4):
                for h in range(2):
                    nc.sync.dma_start(
                        adjc_sb[kc][h][:],
                        adjc_d[kc * 128 : (kc + 1) * 128, h * 8192 : (h + 1) * 8192],
                    )

            def x0_lhs(i):
                """[128, 128] lhsT for layer-0 slot tile i (0-15 own, 16-31 s0)."""
                return x0_sb[i // 16][:, (i % 16) * 128 : (i % 16 + 1) * 128]

            def adj_rhs(kc, i):
                """rhs [128, 512] for contraction slot tile i of output chunk kc."""
                if kc == 0:
                    q, r = divmod(i, 8)
                    return adjc0_sb[q][:, r * 512 : (r + 1) * 512]
                h, r = divmod(i, 16)
                return adjc_sb[kc][h][:, r * 512 : (r + 1) * 512]

            # --- constants (gpsimd queue) ---
            mask_sb = cpool.tile([128, 16], dt.float32, tag="mask")
            nc.gpsimd.dma_start(mask_sb[:], mask_d[:])
            deg_sb = cpool.tile([32, HALF], dt.bfloat16, tag="deg")
            nc.gpsimd.dma_start(deg_sb[:], deg_d[:])
            drow_sb = cpool.tile([32, 256], dt.bfloat16, tag="drow")
            nc.gpsimd.dma_start(drow_sb[:], drow_d[:])
            wa_sb, wb_sb, ba_sb = [], [], []
            for l in range(3):
                wa = cpool.tile([128, 2 * H], dt.bfloat16, tag=f"wa{l}")
                nc.gpsimd.dma_start(wa[:], wa_d[l][:])
                wa_sb.append(wa)
                wb = cpool.tile([2 * H, KC_OUT[l]], dt.bfloat16, tag=f"wb{l}")
                nc.gpsimd.dma_start(wb[:], wb_d[l][:])
                wb_sb.append(wb)
                ba = cpool.tile([128, 1], dt.float32, tag=f"ba{l}")
                nc.gpsimd.dma_start(ba[:], ba_d[l][:])
                ba_sb.append(ba)

            # persistent per-boundary activation tiles
            # xst[bd][kc]: local y = m*z, also the D-group lhsT of layer bd+1
            # s_sb[bd][kc]: AllReduce result y_own + y_peer (P-group lhsT)
            xst = {
                bd: [
                    xstpool.tile([128, 512], dt.bfloat16, tag=f"xst{bd}_{kc}", name=f"xst_{bd}_{kc}")
                    for kc in range(4)
                ]
                for bd in range(n_layers - 1)
            }
            s_sb = {
                bd: [
                    xstpool.tile([128, 512], dt.bfloat16, tag=f"s{bd}_{kc}", name=f"s_{bd}_{kc}")
                    for kc in range(4)
                ]
                for bd in range(n_layers - 1)
            }
            ar_io = {
                bd: (
                    [
                        dpool.tile([128, 512], dt.bfloat16, tag=f"ar_in{c}", name=f"ar_in_{bd}_{c}")
                        for c in range(4)
                    ],
                    [
                        dpool.tile([128, 512], dt.bfloat16, tag=f"ar_out{c}", name=f"ar_out_{bd}_{c}")
                        for c in range(4)
                    ],
                )
                for bd in range(n_layers - 1)
            }
            out_sb = cpool.tile([KC_OUT[2], HALF], dt.float32, tag="out_sb")

            def lhs_tile(l, i):
                """lhsT [128, 128] for layer l, contraction slot tile i."""
                if l == 0:
                    return x0_lhs(i)
                bd = l - 1
                if i < 16:  # D group: local y
                    return xst[bd][i // 4][:, (i % 4) * 128 : (i % 4 + 1) * 128]
                j = i - 16  # P group: AllReduce sum s
                return s_sb[bd][j // 4][:, (j % 4) * 128 : (j % 4 + 1) * 128]

            def emit_agg_group(l, kc, agg_ps, lo, hi, start, stop):
                for i in range(lo, hi):
                    nc.tensor.matmul(
                        agg_ps[:],
                        lhs_tile(l, i),
                        adj_rhs(kc, i),
                        start=(start and i == lo),
                        stop=(stop and i == hi - 1),
                    )

            def emit_rank1(l, kc, agg_ps):
                """agg += deg (x) D_l : the folded mask*bias of the previous
                layer entering this layer's aggregation."""
                bd = l - 1
                nc.tensor.matmul(
                    agg_ps[:],
                    drow_sb[:, bd * 128 : (bd + 1) * 128],
                    deg_sb[:, kc * 512 : (kc + 1) * 512],
                    start=False,
                    stop=True,
                )

            def emit_agg_copy(l, kc, agg_ps):
                agg_sb = wpool.tile([128, 512], dt.bfloat16, tag="agg_sb")
                nc.scalar.copy(agg_sb[:], agg_ps[:])
                return agg_sb

            def emit_mlp1(l, kc, agg_sb):
                h1_ps = ps_mlp.tile([128, 512], dt.float32, tag="h1")
                nc.tensor.matmul(h1_ps[:], wa_sb[l][:], agg_sb[:], start=True, stop=True)
                h1_sb = wpool.tile([128, 512], dt.bfloat16, tag="h1_sb")
                nc.scalar.activation(
                    h1_sb[:],
                    h1_ps[:],
                    mybir.ActivationFunctionType.Relu,
                    bias=ba_sb[l][:, 0:1],
                )
                return h1_sb

            def emit_mlp2(l, kc, h1_sb):
                last = l == n_layers - 1
                if last:
                    # flipped: one [64, 512] matmul; mask+bias on host
                    z2_ps = ps_mlp.tile([KC_OUT[2], 512], dt.float32, tag="xn", bufs=2)
                    nc.tensor.matmul(z2_ps[:], wb_sb[2][:], h1_sb[:], start=True, stop=True)
                    dst = out_sb[:, kc * 512 : (kc + 1) * 512]
                    if kc % 2 == 0:
                        nc.scalar.copy(dst, z2_ps[:])
                    else:
                        nc.vector.tensor_scalar_mul(dst, z2_ps[:], 1.0)
                    nc.sync.dma_start(out_d[:, kc * 512 : (kc + 1) * 512], dst)
                    return
                kco = KC_OUT[l]
                for t in range(4):
                    xn_ps = ps_mlp.tile([128, kco], dt.float32, tag="xn", bufs=2)
                    nc.tensor.matmul(
                        xn_ps[:], h1_sb[:, t * 128 : (t + 1) * 128], wb_sb[l][:],
                        start=True, stop=True,
                    )
                    # y = m * z straight from PSUM, split across scalar/vector
                    # so the 4-tile chain is not serialized on one engine
                    mcol = mask_sb[:, kc * 4 + t : kc * 4 + t + 1]
                    dst = xst[l][kc][:, t * 128 : t * 128 + kco]
                    if t < 2:
                        nc.scalar.activation(
                            dst, xn_ps[:], mybir.ActivationFunctionType.Copy, scale=mcol
                        )
                    else:
                        nc.vector.tensor_scalar_mul(dst, xn_ps[:], mcol)

            def emit_flush(l, kc):
                """Chunk kc's y staged: store + AllReduce + load s."""
                bd = l
                ar_in, ar_out = ar_io[bd]
                nc.sync.dma_start(ar_in[kc][:], xst[bd][kc][:])
                if use_cc:
                    nc.gpsimd.collective_compute(
                        "AllReduce",
                        mybir.AluOpType.add,
                        replica_groups=PAIRS,
                        ins=[ar_in[kc].opt()],
                        outs=[ar_out[kc].opt()],
                    )
                else:
                    nc.sync.dma_start(ar_out[kc][:], ar_in[kc][:])
                # s load on gpsimd right after its own trigger: it only waits
                # on a CC already ahead of the next trigger in the serial CC
                # stream, so it never delays anything.
                nc.gpsimd.dma_start(s_sb[bd][kc][:], ar_out[kc][:])

            def emit_epi(l, kc, agg_sb):
                h1_sb = emit_mlp1(l, kc, agg_sb)
                emit_mlp2(l, kc, h1_sb)

            # --- software-pipelined stage loop ---
            # Layer 0: stream order [D(kc), P(kc)] per chunk; one-chunk skew
            # for the epilogue.  Layers 1-2: all four D-groups first (they
            # only need the previous layer's local xst) -> they cover the
            # last AllReduce's latency; then P-groups + rank-1 in chunk
            # order with the same epilogue skew.
            for l in range(n_layers):
                banks = {}
                if l == 0:
                    pend = None
                    for kc in range(4):
                        agg_ps = ps_agg.tile(
                            [128, 512], dt.float32, tag=f"agg{kc}", name=f"agg_l{l}_{kc}"
                        )
                        emit_agg_group(l, kc, agg_ps, 0, 32, start=True, stop=True)
                        h1_pend = None
                        if pend is not None:
                            h1_pend = emit_mlp1(l, pend[0], pend[1])
                        agg_sb = emit_agg_copy(l, kc, agg_ps)
                        if pend is not None:
                            emit_mlp2(l, pend[0], h1_pend)
                            emit_flush(l, pend[0])
                        pend = (kc, agg_sb)
                    emit_epi(l, 3, pend[1])
                    emit_flush(l, 3)
                    continue
                for kc in range(4):
                    agg_ps = ps_agg.tile(
                        [128, 512], dt.float32, tag=f"agg{kc}", name=f"agg_l{l}_{kc}"
                    )
                    emit_agg_group(l, kc, agg_ps, 0, 16, start=True, stop=False)
                    banks[kc] = agg_ps
                pend = None
                for kc in range(4):
                    emit_agg_group(l, kc, banks[kc], 16, 32, start=False, stop=False)
                    emit_rank1(l, kc, banks[kc])
                    h1_pend = None
                    if pend is not None:
                        h1_pend = emit_mlp1(l, pend[0], pend[1])
                    agg_sb = emit_agg_copy(l, kc, banks[kc])
                    if pend is not None:
                        emit_mlp2(l, pend[0], h1_pend)
                        if l < n_layers - 1:
                            emit_flush(l, pend[0])
                    pend = (kc, agg_sb)
                emit_epi(l, 3, pend[1])
                if l < n_layers - 1:
                    emit_flush(l, 3)

    n_split = _legalize_sync_waits(nc)
    print(f"kernel: legalized {n_split} multi-wait instructions", file=sys.stderr)
    return nc


def get_program():
    if "nc" not in _PROGRAM_CACHE:
        _PROGRAM_CACHE["nc"] = _build_program()
    return _PROGRAM_CACHE["nc"]


def _fold_consts(inputs):
    """Fold BN into weights; return per-layer device consts + host d2."""
    f32 = np.float32
    const = {}
    drows = []
    for l in range(3):
        Wa = np.asarray(inputs[f"Wa{l}"], f32)
        ba = np.asarray(inputs[f"ba{l}"], f32)
        Wb = np.asarray(inputs[f"Wb{l}"], f32)
        bb = np.asarray(inputs[f"bb{l}"], f32)
        s1 = np.asarray(inputs[f"bng{l}"], f32) / np.sqrt(
            np.asarray(inputs[f"bnv{l}"], f32) + BN_EPS
        )
        c1 = np.asarray(inputs[f"bnb{l}"], f32) - np.asarray(inputs[f"bnm{l}"], f32) * s1
        Wb1 = s1[:, None] * Wb
        bb1 = bb + c1 @ Wb
        if l < 2:
            s2 = np.asarray(inputs[f"og{l}"], f32) / np.sqrt(
                np.asarray(inputs[f"ov{l}"], f32) + BN_EPS
            )
            c2 = np.asarray(inputs[f"ob{l}"], f32) - np.asarray(inputs[f"om{l}"], f32) * s2
            Wb2 = (Wb1 * s2[None, :]).astype(f32)
            d = (bb1 * s2 + c2).astype(f32)
            drows.append(np.concatenate([d, d]))
        else:
            Wb2 = Wb1.astype(f32)
            d2 = np.concatenate([bb1, bb1]).astype(f32)  # host-applied
        ci, co = Wa.shape[0], Wb2.shape[1]
        waBD = np.zeros((2 * ci, 2 * H), f32)
        wbBD = np.zeros((2 * H, 2 * co), f32)
        for k in range(2):
            waBD[k * ci : (k + 1) * ci, k * H : (k + 1) * H] = Wa
            wbBD[k * H : (k + 1) * H, k * co : (k + 1) * co] = Wb2
        if l == 0:
            # layer 0: agg psum rows 0:64 = hi part, 64:128 = lo part; stack
            # the 64-row block-diag Wa twice so the MM reduces hi+lo.
            const["wa0"] = np.vstack([waBD, waBD]).astype(BF16)
        else:
            const[f"wa{l}"] = waBD.astype(BF16)
        const[f"wb{l}"] = wbBD.astype(BF16)
        const[f"ba{l}"] = np.concatenate([ba, ba]).reshape(128, 1).astype(f32)
    drow_pad = np.zeros((32, 256), f32)
    drow_pad[0] = np.concatenate(drows)
    const["drow"] = drow_pad.astype(BF16)
    return const, d2


def _pack_hilo(v):
    """[n, 64] f32 -> [n, 128] bf16 [hi | lo]."""
    vh = v.astype(BF16)
    vl = (v - vh.astype(np.float32)).astype(BF16)
    return np.hstack([vh, vl])


def prepare_in_maps(inputs):
    """Host-side prep: fold BN, build per-core D/P adjacency slabs, x0/s0."""
    f32 = np.float32
    x = np.asarray(inputs["x"], f32)
    adj = np.asarray(inputs["adj"], f32)
    mask = np.asarray(inputs["mask"]).astype(bool)

    const, d2 = _fold_consts(inputs)

    in_maps = []
    post = []  # (d2, mask_half) per core for host-side finalization
    for core in range(N_CORES):
        b, h = divmod(core, 2)
        own0 = h * HALF
        peer0 = (1 - h) * HALF
        # Bc[src, out] = adj[b][own0+out, src] + I
        A_rows = adj[b][own0 : own0 + HALF, :]  # [2048 out, 4096 src]
        Bc = np.ascontiguousarray(A_rows.T)
        Bc[np.arange(HALF) + own0, np.arange(HALF)] += 1.0
        D = Bc[own0 : own0 + HALF] - Bc[peer0 : peer0 + HALF]
        P = Bc[peer0 : peer0 + HALF]
        adjX = np.concatenate([D, P], axis=0).astype(FP8)  # [4096 slots, 2048]
        # chunk-major slabs: adjc[kc*128+p, i*512+j] = adjX[i*128+p, kc*512+j]
        adjc = np.ascontiguousarray(
            adjX.reshape(NT, 128, 4, 512).transpose(2, 1, 0, 3).reshape(4 * 128, NT * 512)
        )
        xb = x[b].reshape(N, K * C_IN)
        x_own = xb[own0 : own0 + HALF]
        s0 = x_own + xb[peer0 : peer0 + HALF]
        x0p = np.concatenate([_pack_hilo(x_own), _pack_hilo(s0)], axis=0)  # [4096, 128]
        # block-permuted: x0pp[p, i*128+c] = x0p[i*128+p, c]
        x0pp = np.ascontiguousarray(
            x0p.reshape(NT, 128, 128).transpose(1, 0, 2).reshape(128, NT * 128)
        )
        mvec = mask[b].astype(f32)
        mhalf = mvec[own0 : own0 + HALF]
        deg = A_rows @ mvec + mhalf  # [2048] integer-valued
        deg_pad = np.zeros((32, HALF), f32)
        deg_pad[0] = deg
        m = dict(const)
        m["cc_dummy"] = np.zeros((2, 16), np.float32)
        m["adjc"] = adjc
        m["x0p"] = x0pp
        m["mask_cols"] = np.ascontiguousarray(mhalf.reshape(16, 128).T)
        m["deg_row"] = deg_pad.astype(BF16)
        in_maps.append(m)
        post.append((d2, mhalf))
    return in_maps, post


def assemble_output(res, post):
    """Gather per-core [64, 2048] z2 into the full output, applying the
    final mask and folded bias on host."""
    out = np.zeros((B, N, K, C_OUT), np.float32)
    for core in range(N_CORES):
        b, h = divmod(core, 2)
        r0 = h * HALF
        d2, mhalf = post[core]
        z2 = res.results[core]["out"]  # [64, 2048]
        zt = (z2.T + d2[None, :]) * mhalf[:, None]
        out[b, r0 : r0 + HALF] = zt.reshape(HALF, K, C_OUT)
    return out


def run(in_maps, trace=False, **kw):
    nc = get_program()
    return run_bass_kernel_spmd(nc, in_maps, list(range(N_CORES)), trace=trace, **kw)


def kernel(**inputs) -> np.ndarray:
    in_maps, post = prepare_in_maps(inputs)
    res = run(in_maps)
    return assemble_output(res, post)


# revision 17
# speedup vs baseline: 1.3370x; 1.3370x over previous
"""DenseGIN (3-layer, dense adjacency) Trainium2 Bass kernel, 8-core SPMD. v4.

Problem: x:(4,4096,2,32) f32, adj:(4,4096,4096) f32 binary, mask:(4,4096) bool.
Per layer l: agg = (adj+I) @ xf ; h = relu(agg@Wa+ba); h = BN(h); h = h@Wb+bb;
x = mask*h ; between layers an outer BN is applied at masked nodes.

Sharding: 8 cores = (batch b, node-half). Core (2b+h) owns output nodes
[h*2048,(h+1)*2048) of batch b.

v4 design (on top of v3's fp8 adjacency residency + chunk pipelining):
- The aggregation is linear, so per core
      agg = B_own @ y_own + B_peer @ y_peer
          = (B_own - B_peer) @ y_own + B_peer @ (y_own + y_peer)
  with B = (adj+I).T column slice.  The host bakes D = B_own - B_peer
  (entries {-1,0,1}, exact in fp8e4) and P = B_peer per core, which removes
  the even/odd parity asymmetry from the program: the inter-core exchange
  becomes an AllReduce(add) of y (s = y_own + y_peer), and 16 of the 32
  aggregation matmuls per chunk ("D-group") depend only on locally-computed
  activations.  All four D-groups (64 MMs, ~14us) run at each layer boundary
  while the AllReduces complete -> no PE stall, no HAM p-state drop.
- Mask/bias folding: xst carries y = m (.) z only (single scalar op from
  PSUM).  The dropped +D bias enters the next layer's aggregation as a
  rank-1 update deg (x) D (one K=1 matmul per chunk), with
  deg_j = sum_s (adj+I)[j,s] m_s precomputed on host (integer, bf16-exact).
- Layer 2's MLP2 is flipped (wb stationary, h1 moving): one [64,512] matmul
  per chunk instead of 4 small ones; final mask + bias applied on host.
- All MLP weights/activations in bf16 (fp32 matmuls run at 1/4 rate).
- Layer 0 packs x as [xh|xl] bf16 hi/lo in 128 stationary columns with wa0
  stacked [Wa;Wa] so the hi+lo reduction happens inside the MLP1 matmul;
  the host also prepares s0 = x_own + x_peer hi/lo so layer 0 uses the same
  D/P slab structure.
- Engine queues: sync = bulk input DMA + ar_in/output stores; gpsimd =
  consts + AllReduce triggers + s_sb loads (each load only waits on a CC
  already ahead of the next trigger in the serial CC stream); scalar/vector
  split the epilogue ops.
"""

import sys

if "/opt/trn_rl_repo" not in sys.path:  # PYTHONPATH normally provides it
    sys.path.insert(0, "/opt/trn_rl_repo")

import contextlib
import ctypes
import types

import numpy as np
import ml_dtypes

import concourse.bass as bass
import concourse.tile as tile
from concourse import mybir
from concourse.vector_clock import ScopedClock
import concourse.bass_utils as bass_utils
from concourse.bass_utils import run_bass_kernel_spmd

# ---------------------------------------------------------------------------
# Workaround: the walrus build in this container rejects instructions with
# more than one sem wait ("Too many sync wait commands").  Tile's final drain
# attaches one wait per live semaphore; split them across chained SP drains.
_MAX_WAITS_PER_INST = 1


def _patched_drain_and_barrier(self, tick_clock, wait_clock):
    nc = self.nc
    drain_inst = nc.sync.drain()
    wait_clock.add_sem_waits(drain_inst.ins, ScopedClock({None: tick_clock.global_clock}))
    si = drain_inst.ins.sync_info
    waits = list(si.on_wait or [])
    if len(waits) > _MAX_WAITS_PER_INST:
        si.on_wait = waits[:_MAX_WAITS_PER_INST]
        rest = waits[_MAX_WAITS_PER_INST:]
        for i in range(0, len(rest), _MAX_WAITS_PER_INST):
            extra = nc.sync.drain()
            extra.ins.sync_info = mybir.SyncInfo(
                on_wait=rest[i : i + _MAX_WAITS_PER_INST], on_update=[]
            )
    nc.all_engine_barrier()
    assert self.sems is not None
    popped = nc._tile_sem_poison_stack.pop()
    assert popped is self._sem_poison
    nc.clear_and_free_semaphores(list(self.sems.allocated().values()))
    nc.all_engine_barrier()


tile.TileContext._drain_and_barrier = _patched_drain_and_barrier


def _legalize_sync_waits(nc, max_waits=_MAX_WAITS_PER_INST):
    """Split instructions carrying more than ``max_waits`` sem waits.

    Engine sequencers process their instruction stream in order and execute
    sem waits before dispatch, so hoisting excess waits onto NoOps placed
    just before the instruction (same engine) is semantics-preserving.
    """
    n_split = 0
    for fn in nc.m.functions:
        for blk in fn.blocks:
            insts = blk.instructions
            i = 0
            while i < len(insts):
                inst = insts[i]
                si = inst.sync_info
                waits = list(si.on_wait) if si and si.on_wait else []
                if len(waits) > max_waits:
                    extra, keep = waits[:-max_waits], waits[-max_waits:]
                    si.on_wait = keep
                    pos = i
                    for j in range(0, len(extra), max_waits):
                        nop = mybir.InstNoOp(name=f"I-lsw{n_split}-{j}", ins=[], outs=[])
                        nop.engine = inst.engine
                        nop.sync_info = mybir.SyncInfo(
                            on_wait=extra[j : j + max_waits], on_update=[]
                        )
                        insts.insert(pos, nop)
                        pos += 1
                        i += 1
                    n_split += 1
                i += 1
    return n_split


# ---------------------------------------------------------------------------
# NTFF profiling hook (antenv.axon_hooks is absent in this image).  Only used
# when run() is called with trace=True; registering it is harmless otherwise.
def _ntff_profile_via_ctypes(so_path):
    try:
        lib = ctypes.CDLL(so_path)
    except OSError:
        return None
    if not hasattr(lib, "axon_start_nrt_profile"):
        return None
    lib.axon_start_nrt_profile.argtypes = [ctypes.POINTER(ctypes.c_int64), ctypes.c_size_t]
    lib.axon_start_nrt_profile.restype = ctypes.c_int64
    lib.axon_stop_nrt_profile.argtypes = [ctypes.c_char_p]
    lib.axon_stop_nrt_profile.restype = ctypes.c_int64

    @contextlib.contextmanager
    def _hook(output_dir, device_ids):
        import jax

        jax.devices()
        if device_ids:
            ids = (ctypes.c_int64 * len(device_ids))(*device_ids)
            rc = lib.axon_start_nrt_profile(ids, len(device_ids))
        else:
            rc = lib.axon_start_nrt_profile(None, 0)
        if rc != 0:
            raise RuntimeError(f"axon_start_nrt_profile rc={rc}")
        try:
            yield
        finally:
            n = lib.axon_stop_nrt_profile(str(output_dir).encode())
            print(f"ntff profile: {n} file(s) written to {output_dir}", file=sys.stderr)

    return _hook


if "antenv.axon_hooks" not in sys.modules:
    _hooks_mod = types.ModuleType("antenv.axon_hooks")
    _hook_inst = _ntff_profile_via_ctypes("/opt/axon/libaxon_pjrt.so")
    _hooks_mod.get_axon_ntff_profile_hook = lambda: _hook_inst
    sys.modules["antenv.axon_hooks"] = _hooks_mod
bass_utils.upload_artifacts = lambda tmpdir: f"local:{tmpdir}"

# ---------------------------------------------------------------------------
B, N, K, C_IN, H, C_OUT = 4, 4096, 2, 32, 64, 32
BN_EPS = 1e-5
N_CORES = 8
HALF = N // 2          # 2048 output nodes per core
NT = 32                # 32 contraction slot tiles: 16 D (own) + 16 P (peer/sum)
KC_OUT = [K * H, K * H, K * C_OUT]   # flat output channels per layer: 128,128,64

BF16 = ml_dtypes.bfloat16
FP8 = ml_dtypes.float8_e4m3  # == mybir float8e4 (TRN FP8_EXP4); {-1,0,1} exact

PAIRS = [[0, 1], [2, 3], [4, 5], [6, 7]]

_PROGRAM_CACHE = {}


def _build_program(n_layers=3, use_cc=True):
    """Build the SPMD Bass/Tile program (identical on all 8 cores)."""
    nc = bass.Bass("TRN2", target_bir_lowering=False, debug=False, num_devices=N_CORES)
    dt = mybir.dt

    # chunk-major adjacency slabs: row kc*128+p, col i*512+j =
    #   D[i*128+p, kc*512+j]          for i < 16   (D = B_own - B_peer)
    #   P[(i-16)*128+p, kc*512+j]     for i >= 16  (P = B_peer)
    # Chunk kc's slab is one contiguous [128, 16384] block, D half first.
    adjc_d = nc.dram_tensor("adjc", [4 * 128, NT * 512], dt.float8e4, kind="ExternalInput").ap()
    # layer-0 x, packed [xh | xl] bf16, block-permuted: col i*128+c = slot-tile
    # i's packed channel c.  Slot tiles 0-15 = own x, 16-31 = s0 = x_own+x_peer.
    x0_d = nc.dram_tensor("x0p", [128, NT * 128], dt.bfloat16, kind="ExternalInput").ap()
    mask_d = nc.dram_tensor("mask_cols", [128, 16], dt.float32, kind="ExternalInput").ap()
    # deg/drow are logically 1-row; zero-padded to K=32 partitions so the
    # rank-1 matmul runs as a standard 32-row tile (no reliance on HW
    # zero-padding of sub-tile moving operands).
    deg_d = nc.dram_tensor("deg_row", [32, HALF], dt.bfloat16, kind="ExternalInput").ap()
    drow_d = nc.dram_tensor("drow", [32, 256], dt.bfloat16, kind="ExternalInput").ap()
    dummy_d = nc.dram_tensor("cc_dummy", [2, 16], dt.float32, kind="ExternalInput").ap()
    # Wa/Wb are stored block-diagonally over the K=2 slice structure so each
    # MLP stage is a single full-partition matmul with base_partition 0
    # (partition-offset matmul operands crash at runtime on this stack).
    # wa0 additionally stacks the block twice (hi/lo reduction in the MM).
    wa_d = [
        nc.dram_tensor(f"wa{l}", [128, 2 * H], dt.bfloat16, kind="ExternalInput").ap()
        for l in range(3)
    ]
    wb_d = [
        nc.dram_tensor(f"wb{l}", [2 * H, KC_OUT[l]], dt.bfloat16, kind="ExternalInput").ap()
        for l in range(3)
    ]
    ba_d = [
        nc.dram_tensor(f"ba{l}", [128, 1], dt.float32, kind="ExternalInput").ap()
        for l in range(3)
    ]
    # final output: flipped layout [flat channel, node] (host transposes)
    out_d = nc.dram_tensor("out", [KC_OUT[2], HALF], dt.float32, kind="ExternalOutput").ap()

    with tile.TileContext(nc) as tc:
        with (
            tc.tile_pool(name="const", bufs=1) as cpool,
            tc.tile_pool(name="xst", bufs=1) as xstpool,
            tc.tile_pool(name="work", bufs=3) as wpool,
            tc.tile_pool(name="ps_agg", bufs=1, space="PSUM") as ps_agg,
            tc.tile_pool(name="ps_mlp", bufs=2, space="PSUM") as ps_mlp,
            tc.tile_pool(name="dram", bufs=2, space="DRAM") as dpool,
        ):
            # --- HAM warmup: dummy matmuls prime the PE clock toward 8/8
            # while the head DMAs stream ---
            wu_lhs = cpool.tile([128, 128], dt.bfloat16, tag="wu_lhs")
            wu_rhs = cpool.tile([128, 512], dt.bfloat16, tag="wu_rhs")
            nc.gpsimd.memset(wu_lhs[:], 0.0)
            nc.gpsimd.memset(wu_rhs[:], 0.0)
            wu_ps = ps_mlp.tile([128, 512], dt.float32, tag="h1")
            for _ in range(8):
                nc.tensor.matmul(wu_ps[:], wu_lhs[:], wu_rhs[:], start=True, stop=True)

            # --- dummy AllReduce first: every core reaches this trigger
            # within ~1us of launch (no data deps), so the one-time CC
            # bootstrap/rendezvous runs against launch skew instead of
            # layer-0 compute progress ---
            if use_cc:
                dummy_in = dpool.tile([2, 16], dt.float32, tag="cc_warm_in", bufs=1)
                dummy_out = dpool.tile([4, 16], dt.float32, tag="cc_warm_out", bufs=1)
                nc.gpsimd.dma_start(dummy_in[:], dummy_d[:])
                nc.gpsimd.collective_compute(
                    "AllGather",
                    mybir.AluOpType.bypass,
                    replica_groups=PAIRS,
                    ins=[dummy_in.opt()],
                    outs=[dummy_out.opt()],
                )

            # --- bulk inputs on the sync queue in consumption order:
            # x0-own, chunk-0 D half (quarters), x0-s, chunk-0 P half
            # (quarters), then chunks 1-3 as D/P halves ---
            x0_sb = [
                cpool.tile([128, 16 * 128], dt.bfloat16, tag=f"x0h{h}", name=f"x0_{h}")
                for h in range(2)
            ]
            adjc0_sb = [
                cpool.tile([128, 8 * 512], dt.float8e4, tag=f"adjc0q{q}", name=f"adjc0_{q}")
                for q in range(4)
            ]
            adjc_sb = {
                kc: [
                    cpool.tile([128, 16 * 512], dt.float8e4, tag=f"adjc{kc}h{h}", name=f"adjc_{kc}_{h}")
                    for h in range(2)
                ]
                for kc in range(1, 4)
            }
            nc.sync.dma_start(x0_sb[0][:], x0_d[:, 0:2048])
            for q in range(2):
                nc.sync.dma_start(adjc0_sb[q][:], adjc_d[0:128, q * 4096 : (q + 1) * 4096])
            nc.sync.dma_start(x0_sb[1][:], x0_d[:, 2048:4096])
            for q in range(2, 4):
                nc.sync.dma_start(adjc0_sb[q][:], adjc_d[0:128, q * 4096 : (q + 1) * 4096])
            for kc in range(1, 4):
                for h in range(2):
                    nc.sync.dma_start(
                        adjc_sb[kc][h][:],
                        adjc_d[kc * 128 : (kc + 1) * 128, h * 8192 : (h + 1) * 8192],
                    )

            def x0_lhs(i):
                """[128, 128] lhsT for layer-0 slot tile i (0-15 own, 16-31 s0)."""
                return x0_sb[i // 16][:, (i % 16) * 128 : (i % 16 + 1) * 128]

            def adj_rhs(kc, i):
                """rhs [128, 512] for contraction slot tile i of output chunk kc."""
                if kc == 0:
                    q, r = divmod(i, 8)
                    return adjc0_sb[q][:, r * 512 : (r + 1) * 512]
                h, r = divmod(i, 16)
                return adjc_sb[kc][h][:, r * 512 : (r + 1) * 512]

            # --- constants (gpsimd queue) ---
            mask_sb = cpool.tile([128, 16], dt.float32, tag="mask")
            nc.gpsimd.dma_start(mask_sb[:], mask_d[:])
            deg_sb = cpool.tile([32, HALF], dt.bfloat16, tag="deg")
            nc.gpsimd.dma_start(deg_sb[:], deg_d[:])
            drow_sb = cpool.tile([32, 256], dt.bfloat16, tag="drow")
            nc.gpsimd.dma_start(drow_sb[:], drow_d[:])
            wa_sb, wb_sb, ba_sb = [], [], []
            for l in range(3):
                wa = cpool.tile([128, 2 * H], dt.bfloat16, tag=f"wa{l}")
                nc.gpsimd.dma_start(wa[:], wa_d[l][:])
                wa_sb.append(wa)
                wb = cpool.tile([2 * H, KC_OUT[l]], dt.bfloat16, tag=f"wb{l}")
                nc.gpsimd.dma_start(wb[:], wb_d[l][:])
                wb_sb.append(wb)
                ba = cpool.tile([128, 1], dt.float32, tag=f"ba{l}")
                nc.gpsimd.dma_start(ba[:], ba_d[l][:])
                ba_sb.append(ba)

            # persistent per-boundary activation tiles
            # xst[bd][kc]: local y = m*z, also the D-group lhsT of layer bd+1
            # s_sb[bd]: y_own + y_peer (P-group lhsT), slot tile j at cols
            # j*128 (chunks side by side)
            xst = {
                bd: [
                    xstpool.tile([128, 512], dt.bfloat16, tag=f"xst{bd}_{kc}", name=f"xst_{bd}_{kc}")
                    for kc in range(4)
                ]
                for bd in range(n_layers - 1)
            }
            s_sb = {
                bd: xstpool.tile([128, HALF], dt.bfloat16, tag=f"s{bd}", name=f"s_{bd}")
                for bd in range(n_layers - 1)
            }
            # exchange in chunk-pair halves: ag_in [128,1024] -> ag_out [256,1024]
            ag_io = {
                bd: (
                    [
                        dpool.tile([128, 1024], dt.bfloat16, tag=f"ag_in{h}", name=f"ag_in_{bd}_{h}")
                        for h in range(2)
                    ],
                    [
                        dpool.tile([256, 1024], dt.bfloat16, tag=f"ag_out{h}", name=f"ag_out_{bd}_{h}")
                        for h in range(2)
                    ],
                )
                for bd in range(n_layers - 1)
            }
            out_sb = cpool.tile([KC_OUT[2], HALF], dt.float32, tag="out_sb")

            def lhs_tile(l, i):
                """lhsT [128, 128] for layer l, contraction slot tile i."""
                if l == 0:
                    return x0_lhs(i)
                bd = l - 1
                if i < 16:  # D group: local y
                    return xst[bd][i // 4][:, (i % 4) * 128 : (i % 4 + 1) * 128]
                j = i - 16  # P group: exchanged sum s
                return s_sb[bd][:, j * 128 : (j + 1) * 128]

            def emit_agg_group(l, kc, agg_ps, lo, hi, start, stop):
                for i in range(lo, hi):
                    nc.tensor.matmul(
                        agg_ps[:],
                        lhs_tile(l, i),
                        adj_rhs(kc, i),
                        start=(start and i == lo),
                        stop=(stop and i == hi - 1),
                    )

            def emit_rank1(l, kc, agg_ps):
                """agg += deg (x) D_l : the folded mask*bias of the previous
                layer entering this layer's aggregation."""
                bd = l - 1
                nc.tensor.matmul(
                    agg_ps[:],
                    drow_sb[:, bd * 128 : (bd + 1) * 128],
                    deg_sb[:, kc * 512 : (kc + 1) * 512],
                    start=False,
                    stop=False,
                )

            def emit_agg_copy(l, kc, agg_ps):
                agg_sb = wpool.tile([128, 512], dt.bfloat16, tag="agg_sb")
                nc.scalar.copy(agg_sb[:], agg_ps[:])
                return agg_sb

            def emit_mlp1(l, kc, agg_sb):
                h1_ps = ps_mlp.tile([128, 512], dt.float32, tag="h1")
                nc.tensor.matmul(h1_ps[:], wa_sb[l][:], agg_sb[:], start=True, stop=True)
                h1_sb = wpool.tile([128, 512], dt.bfloat16, tag="h1_sb")
                nc.scalar.activation(
                    h1_sb[:],
                    h1_ps[:],
                    mybir.ActivationFunctionType.Relu,
                    bias=ba_sb[l][:, 0:1],
                )
                return h1_sb

            def emit_mlp2(l, kc, h1_sb):
                last = l == n_layers - 1
                if last:
                    # flipped: one [64, 512] matmul; mask+bias on host
                    z2_ps = ps_mlp.tile([KC_OUT[2], 512], dt.float32, tag="xn", bufs=2)
                    nc.tensor.matmul(z2_ps[:], wb_sb[2][:], h1_sb[:], start=True, stop=True)
                    dst = out_sb[:, kc * 512 : (kc + 1) * 512]
                    if kc % 2 == 0:
                        nc.scalar.copy(dst, z2_ps[:])
                    else:
                        nc.vector.tensor_scalar_mul(dst, z2_ps[:], 1.0)
                    nc.sync.dma_start(out_d[:, kc * 512 : (kc + 1) * 512], dst)
                    return
                kco = KC_OUT[l]
                for t in range(4):
                    xn_ps = ps_mlp.tile([128, kco], dt.float32, tag="xn", bufs=2)
                    nc.tensor.matmul(
                        xn_ps[:], h1_sb[:, t * 128 : (t + 1) * 128], wb_sb[l][:],
                        start=True, stop=True,
                    )
                    # y = m * z straight from PSUM, split across scalar/vector
                    # so the 4-tile chain is not serialized on one engine
                    mcol = mask_sb[:, kc * 4 + t : kc * 4 + t + 1]
                    dst = xst[l][kc][:, t * 128 : t * 128 + kco]
                    if t < 2:
                        nc.scalar.activation(
                            dst, xn_ps[:], mybir.ActivationFunctionType.Copy, scale=mcol
                        )
                    else:
                        nc.vector.tensor_scalar_mul(dst, xn_ps[:], mcol)

            def emit_trigger(bd, h):
                ag_in, ag_out = ag_io[bd]
                if use_cc:
                    nc.gpsimd.collective_compute(
                        "AllGather",
                        mybir.AluOpType.bypass,
                        replica_groups=PAIRS,
                        ins=[ag_in[h].opt()],
                        outs=[ag_out[h].opt()],
                    )
                else:
                    nc.sync.dma_start(ag_out[h][0:128, :], ag_in[h][:])
                    nc.sync.dma_start(ag_out[h][128:256, :], ag_in[h][:])

            def emit_gather_s(bd):
                """Per-chunk loads + adds of both AG halves -> s_sb[bd].
                All on gpsimd: each op only waits on a collective already
                ahead of any later trigger in the serial CC stream, and
                gpsimd is otherwise idle during layer bodies."""
                ag_out = ag_io[bd][1]
                for kc in range(4):
                    h, c = divmod(kc, 2)
                    cs = slice(c * 512, (c + 1) * 512)
                    ga = wpool.tile([128, 512], dt.bfloat16, tag="ag_lda", bufs=2)
                    gb = wpool.tile([128, 512], dt.bfloat16, tag="ag_ldb", bufs=2)
                    nc.gpsimd.dma_start(ga[:], ag_out[h][0:128, cs])
                    nc.gpsimd.dma_start(gb[:], ag_out[h][128:256, cs])
                    nc.gpsimd.tensor_add(
                        s_sb[bd][:, kc * 512 : (kc + 1) * 512], ga[:], gb[:]
                    )

            def emit_flush(l, kc):
                """Chunk kc's y staged into its exchange half; trigger on the
                odd chunk of each half; loads+adds once the last store is in."""
                bd = l
                ag_in = ag_io[bd][0]
                nc.sync.dma_start(
                    ag_in[kc // 2][:, (kc % 2) * 512 : (kc % 2 + 1) * 512],
                    xst[bd][kc][:],
                )
                if kc == 1:
                    emit_trigger(bd, 0)
                elif kc == 3:
                    emit_trigger(bd, 1)
                    emit_gather_s(bd)

            def emit_epi(l, kc, agg_sb):
                h1_sb = emit_mlp1(l, kc, agg_sb)
                emit_mlp2(l, kc, h1_sb)

            # --- software-pipelined stage loop ---
            # Layer 0: stream order [D(kc), P(kc)] per chunk; one-chunk skew
            # for the epilogue.  Layers 1-2: all four D-groups first (they
            # only need the previous layer's local xst) -> they cover the
            # last AllReduce's latency; then P-groups + rank-1 in chunk
            # order with the same epilogue skew.
            for l in range(n_layers):
                banks = {}
                if l == 0:
                    pend = None
                    for kc in range(4):
                        agg_ps = ps_agg.tile(
                            [128, 512], dt.float32, tag=f"agg{kc}", name=f"agg_l{l}_{kc}"
                        )
                        emit_agg_group(l, kc, agg_ps, 0, 32, start=True, stop=True)
                        h1_pend = None
                        if pend is not None:
                            h1_pend = emit_mlp1(l, pend[0], pend[1])
                        agg_sb = emit_agg_copy(l, kc, agg_ps)
                        if pend is not None:
                            emit_mlp2(l, pend[0], h1_pend)
                            emit_flush(l, pend[0])
                        pend = (kc, agg_sb)
                    emit_epi(l, 3, pend[1])
                    emit_flush(l, 3)
                    continue
                # phase A: D-groups + rank-1 (no collective dependency) cover
                # the boundary exchange; phase B consumes s chunks 0-1 (first
                # AG half); phase C consumes s chunks 2-3 with the epilogue
                # skew.
                for kc in range(4):
                    agg_ps = ps_agg.tile(
                        [128, 512], dt.float32, tag=f"agg{kc}", name=f"agg_l{l}_{kc}"
                    )
                    emit_agg_group(l, kc, agg_ps, 0, 16, start=True, stop=False)
                    emit_rank1(l, kc, agg_ps)
                    banks[kc] = agg_ps
                for kc in range(4):
                    emit_agg_group(l, kc, banks[kc], 16, 24, start=False, stop=False)
                pend = None
                for kc in range(4):
                    emit_agg_group(l, kc, banks[kc], 24, 32, start=False, stop=True)
                    h1_pend = None
                    if pend is not None:
                        h1_pend = emit_mlp1(l, pend[0], pend[1])
                    agg_sb = emit_agg_copy(l, kc, banks[kc])
                    if pend is not None:
                        emit_mlp2(l, pend[0], h1_pend)
                        if l < n_layers - 1:
                            emit_flush(l, pend[0])
                    pend = (kc, agg_sb)
                emit_epi(l, 3, pend[1])
                if l < n_layers - 1:
                    emit_flush(l, 3)

    n_split = _legalize_sync_waits(nc)
    print(f"kernel: legalized {n_split} multi-wait instructions", file=sys.stderr)
    return nc


def get_program():
    if "nc" not in _PROGRAM_CACHE:
        _PROGRAM_CACHE["nc"] = _build_program()
    return _PROGRAM_CACHE["nc"]


def _fold_consts(inputs):
    """Fold BN into weights; return per-layer device consts + host d2."""
    f32 = np.float32
    const = {}
    drows = []
    for l in range(3):
        Wa = np.asarray(inputs[f"Wa{l}"], f32)
        ba = np.asarray(inputs[f"ba{l}"], f32)
        Wb = np.asarray(inputs[f"Wb{l}"], f32)
        bb = np.asarray(inputs[f"bb{l}"], f32)
        s1 = np.asarray(inputs[f"bng{l}"], f32) / np.sqrt(
            np.asarray(inputs[f"bnv{l}"], f32) + BN_EPS
        )
        c1 = np.asarray(inputs[f"bnb{l}"], f32) - np.asarray(inputs[f"bnm{l}"], f32) * s1
        Wb1 = s1[:, None] * Wb
        bb1 = bb + c1 @ Wb
        if l < 2:
            s2 = np.asarray(inputs[f"og{l}"], f32) / np.sqrt(
                np.asarray(inputs[f"ov{l}"], f32) + BN_EPS
            )
            c2 = np.asarray(inputs[f"ob{l}"], f32) - np.asarray(inputs[f"om{l}"], f32) * s2
            Wb2 = (Wb1 * s2[None, :]).astype(f32)
            d = (bb1 * s2 + c2).astype(f32)
            drows.append(np.concatenate([d, d]))
        else:
            Wb2 = Wb1.astype(f32)
            d2 = np.concatenate([bb1, bb1]).astype(f32)  # host-applied
        ci, co = Wa.shape[0], Wb2.shape[1]
        waBD = np.zeros((2 * ci, 2 * H), f32)
        wbBD = np.zeros((2 * H, 2 * co), f32)
        for k in range(2):
            waBD[k * ci : (k + 1) * ci, k * H : (k + 1) * H] = Wa
            wbBD[k * H : (k + 1) * H, k * co : (k + 1) * co] = Wb2
        if l == 0:
            # layer 0: agg psum rows 0:64 = hi part, 64:128 = lo part; stack
            # the 64-row block-diag Wa twice so the MM reduces hi+lo.
            const["wa0"] = np.vstack([waBD, waBD]).astype(BF16)
        else:
            const[f"wa{l}"] = waBD.astype(BF16)
        const[f"wb{l}"] = wbBD.astype(BF16)
        const[f"ba{l}"] = np.concatenate([ba, ba]).reshape(128, 1).astype(f32)
    drow_pad = np.zeros((32, 256), f32)
    drow_pad[0] = np.concatenate(drows)
    const["drow"] = drow_pad.astype(BF16)
    return const, d2


def _pack_hilo(v):
    """[n, 64] f32 -> [n, 128] bf16 [hi | lo]."""
    vh = v.astype(BF16)
    vl = (v - vh.astype(np.float32)).astype(BF16)
    return np.hstack([vh, vl])


def prepare_in_maps(inputs):
    """Host-side prep: fold BN, build per-core D/P adjacency slabs, x0/s0."""
    f32 = np.float32
    x = np.asarray(inputs["x"], f32)
    adj = np.asarray(inputs["adj"], f32)
    mask = np.asarray(inputs["mask"]).astype(bool)

    const, d2 = _fold_consts(inputs)

    in_maps = []
    post = []  # (d2, mask_half) per core for host-side finalization
    for core in range(N_CORES):
        b, h = divmod(core, 2)
        own0 = h * HALF
        peer0 = (1 - h) * HALF
        # Bc[src, out] = adj[b][own0+out, src] + I
        A_rows = adj[b][own0 : own0 + HALF, :]  # [2048 out, 4096 src]
        Bc = np.ascontiguousarray(A_rows.T)
        Bc[np.arange(HALF) + own0, np.arange(HALF)] += 1.0
        D = Bc[own0 : own0 + HALF] - Bc[peer0 : peer0 + HALF]
        P = Bc[peer0 : peer0 + HALF]
        adjX = np.concatenate([D, P], axis=0).astype(FP8)  # [4096 slots, 2048]
        # chunk-major slabs: adjc[kc*128+p, i*512+j] = adjX[i*128+p, kc*512+j]
        adjc = np.ascontiguousarray(
            adjX.reshape(NT, 128, 4, 512).transpose(2, 1, 0, 3).reshape(4 * 128, NT * 512)
        )
        xb = x[b].reshape(N, K * C_IN)
        x_own = xb[own0 : own0 + HALF]
        s0 = x_own + xb[peer0 : peer0 + HALF]
        x0p = np.concatenate([_pack_hilo(x_own), _pack_hilo(s0)], axis=0)  # [4096, 128]
        # block-permuted: x0pp[p, i*128+c] = x0p[i*128+p, c]
        x0pp = np.ascontiguousarray(
            x0p.reshape(NT, 128, 128).transpose(1, 0, 2).reshape(128, NT * 128)
        )
        mvec = mask[b].astype(f32)
        mhalf = mvec[own0 : own0 + HALF]
        deg = A_rows @ mvec + mhalf  # [2048] integer-valued
        deg_pad = np.zeros((32, HALF), f32)
        deg_pad[0] = deg
        m = dict(const)
        m["cc_dummy"] = np.zeros((2, 16), np.float32)
        m["adjc"] = adjc
        m["x0p"] = x0pp
        m["mask_cols"] = np.ascontiguousarray(mhalf.reshape(16, 128).T)
        m["deg_row"] = deg_pad.astype(BF16)
        in_maps.append(m)
        post.append((d2, mhalf))
    return in_maps, post


def assemble_output(res, post):
    """Gather per-core [64, 2048] z2 into the full output, applying the
    final mask and folded bias on host."""
    out = np.zeros((B, N, K, C_OUT), np.float32)
    for core in range(N_CORES):
        b, h = divmod(core, 2)
        r0 = h * HALF
        d2, mhalf = post[core]
        z2 = res.results[core]["out"]  # [64, 2048]
        zt = (z2.T + d2[None, :]) * mhalf[:, None]
        out[b, r0 : r0 + HALF] = zt.reshape(HALF, K, C_OUT)
    return out


def run(in_maps, trace=False, **kw):
    nc = get_program()
    return run_bass_kernel_spmd(nc, in_maps, list(range(N_CORES)), trace=trace, **kw)


def kernel(**inputs) -> np.ndarray:
    in_maps, post = prepare_in_maps(inputs)
    res = run(in_maps)
    return assemble_output(res, post)
